# revision 2
# baseline (speedup 1.0000x reference)
"""BLT model TRN2 kernel v3 — nn_BLTModel_13872744366807.

v3 over v2:
- Two-stream software pipeline: stream A = patch tokens 0:128, stream B =
  128:256. Causal attention makes A fully independent of B (B reads A's
  K/V). Each AllReduce flies under the other stream's sublayer compute.
- Biases via host-replicated fp16 tiles fused into the PSUM drain add
  (v2's outer-product bias matmuls cost 60+us of PE).
- fp16 logits AllReduce.
Everything else as v2: DP-2 x TP-4, vocab collapse, fp16 compute with
fp32 PSUM, direct LN with host-folded affines, wide-PSUM GEMMs.
"""
import numpy as np
import concourse.bacc as bacc
import concourse.bass as bass
import concourse.mybir as mybir
from concourse import tile
from concourse.bass_utils import run_bass_kernel_spmd
from concourse.bass_interp import get_hw_module

F32 = mybir.dt.float32
FP16 = mybir.dt.float16
AF = mybir.ActivationFunctionType
ALU = mybir.AluOpType

L, B, S, P, H, V, NC = 4, 2, 4096, 256, 1024, 256, 8
EPS = 1e-6
RG4 = [[0, 1, 2, 3], [4, 5, 6, 7]]
T0 = slice(0, 128)
T1 = slice(128, 256)

_CACHE = {}


def _trace(skip_kvn_ln):
    nc = bacc.Bacc("TRN2", target_bir_lowering=False, debug=False,
                   num_devices=NC)
    d = {}

    def inp(name, shape, dt=FP16):
        d[name] = nc.dram_tensor(name, shape, dt, kind="ExternalInput").ap()

    inp("wqkv", [L, 128, 6144])
    inp("wo", [L, 128, 2048])
    inp("w1", [L, 128, 8192])
    inp("w2", [L, 128, 8192])
    # replicated fp16 bias tiles: [0:768) qkv, [768:1792) wo, [1792:2816) w2
    inp("lb", [L, 128, 2816])
    # w1 bias row (outer-product stationary; added pre-Gelu)
    inp("w1r", [L, 1, 1024])
    # replicated CA q/k/v biases: [p, {q0,q1,k0,k1,v0,v1}, 256]
    inp("cabR", [128, 1536])
    inp("cnt", [128, 512])
    inp("embS", [128, 2048])
    inp("embT", [128, 2048])
    inp("masks", [128, 512])
    inp("ones", [128, 128])
    inp("ident", [128, 128])
    inp("wq", [128, 2048]); inp("wk", [128, 2048]); inp("wv", [128, 2048])
    inp("cawoT", [128, 2048])
    inp("headw", [128, 2048])
    # f32 cols: [0:2) headb, [2:10) fng, [10:18) fnb
    inp("cbias", [128, 18], F32)
    out_d = nc.dram_tensor("ltab", [128, 512], F32, kind="ExternalOutput").ap()

    with tile.TileContext(nc) as tc:
        with (
            tc.tile_pool(name="const", bufs=1) as cp,
            tc.tile_pool(name="sb", bufs=1) as sbp,
            tc.tile_pool(name="wts", bufs=2) as wp,
            tc.tile_pool(name="tmp", bufs=2) as tp,
            tc.tile_pool(name="tps", bufs=1) as tps,
            tc.tile_pool(name="mm", bufs=2, space="PSUM") as pmm,
            tc.tile_pool(name="att", bufs=2, space="PSUM") as pat,
            tc.tile_pool(name="ptr", bufs=1, space="PSUM") as ptr,
            tc.tile_pool(name="st", bufs=1, space="PSUM") as pst,
            tc.tile_pool(name="dram", bufs=1, space="DRAM") as dp,
        ):
            # ---------------- constants ----------------
            def cload(name, shape, dt=FP16):
                t_ = cp.tile(shape, dt, tag=name)
                nc.sync.dma_start(t_[:], d[name][:])
                return t_

            embS_t = cp.tile([128, 2, 1024], FP16, tag="embS")
            nc.sync.dma_start(embS_t[:], d["embS"][:].rearrange(
                "p (kc x) -> p kc x", kc=2))
            cnt_t = cp.tile([128, 2, 256], FP16, tag="cnt")
            nc.sync.dma_start(cnt_t[:], d["cnt"][:].rearrange(
                "p (kc x) -> p kc x", kc=2))
            ones_t = cload("ones", [128, 128])
            ident_t = cload("ident", [128, 128])
            masks_t = cload("masks", [128, 512])
            cbias_t = cload("cbias", [128, 18], F32)
            cabR_t = cp.tile([128, 6, 256], FP16, tag="cabR")
            nc.sync.dma_start(cabR_t[:], d["cabR"][:].rearrange(
                "p (kc x) -> p kc x", kc=6))
            embT_t = cp.tile([128, 8, 256], FP16, tag="embT")
            nc.sync.dma_start(embT_t[:], d["embT"][:].rearrange(
                "p (kc x) -> p kc x", kc=8))
            wq_t = cp.tile([128, 8, 256], FP16, tag="wq")
            nc.sync.dma_start(wq_t[:], d["wq"][:].rearrange(
                "p (kc x) -> p kc x", kc=8))
            wk_t = cp.tile([128, 8, 256], FP16, tag="wk")
            nc.sync.dma_start(wk_t[:], d["wk"][:].rearrange(
                "p (kc x) -> p kc x", kc=8))
            wv_t = cp.tile([128, 8, 256], FP16, tag="wv")
            nc.sync.dma_start(wv_t[:], d["wv"][:].rearrange(
                "p (kc x) -> p kc x", kc=8))
            cawoT_t = cp.tile([128, 8, 256], FP16, tag="cawoT")
            nc.sync.dma_start(cawoT_t[:], d["cawoT"][:].rearrange(
                "p (kc x) -> p kc x", kc=8))
            headw_t = cp.tile([128, 8, 256], FP16, tag="headw")
            nc.sync.dma_start(headw_t[:], d["headw"][:].rearrange(
                "p (kc x) -> p kc x", kc=8))

            # cc warm-up: tiny AllReduce early
            wbin = dp.tile([128, 8], F32, tag="wrmi")
            wbout = dp.tile([128, 8], F32, tag="wrmo")
            nc.sync.dma_start(wbin[:], d["cbias"][:, 0:8])
            nc.gpsimd.collective_compute(
                "AllReduce", ALU.add, replica_groups=RG4,
                ins=[wbin[:].opt()], outs=[wbout[:].opt()])

            # ---------------- persistent activations ----------------
            h_t = sbp.tile([128, 8, 256], FP16, tag="h")
            n_t = sbp.tile([128, 8, 256], FP16, tag="n")
            sq_t = sbp.tile([128, 8, 128], FP16, tag="sq")
            st1_t = sbp.tile([128, 4, 128], FP16, tag="st1")
            st2_t = sbp.tile([128, 2, 128], FP16, tag="st2")
            su1_t = sbp.tile([128, 4, 128], FP16, tag="su1")
            su2_t = sbp.tile([128, 2, 128], FP16, tag="su2")
            stsu_t = sbp.tile([128, 2, 128], FP16, tag="stsu")
            rm16_t = sbp.tile([1, 2, 128], FP16, tag="rm16")
            qkv_t = sbp.tile([128, 6, 256], FP16, tag="qkv")
            qkvh2_t = sbp.tile([64, 6, 256], FP16, tag="qkvh2")
            vtok_t = sbp.tile([128, 4, 2, 64], FP16, tag="vtok")
            A_t = sbp.tile([128, 2, 256], FP16, tag="A")
            gu_t = sbp.tile([128, 8, 256], FP16, tag="gu")
            aro_t = sbp.tile([128, 2, 8, 128], FP16, tag="aro")
            ari_t = sbp.tile([128, 2, 8, 128], FP16, tag="ari")
            aro2_t = sbp.tile([128, 2, 8, 128], FP16, tag="aro2")
            ari2_t = sbp.tile([128, 2, 8, 128], FP16, tag="ari2")
            qn_t = sbp.tile([128, 8, 256], FP16, tag="qn")
            qT_t = sbp.tile([128, 2, 256], FP16, tag="qT")
            kT_t = sbp.tile([128, 2, 256], FP16, tag="kT")
            vT_t = sbp.tile([128, 2, 256], FP16, tag="vT")
            O_t = sbp.tile([128, 2, 256], FP16, tag="O")
            w2c_t = sbp.tile([128, 2, 256], FP16, tag="w2c")
            et_t = sbp.tile([128, 2, 256], F32, tag="et")

            # ---------------- helpers ----------------
            def stats_half(src, cs):
                """LN stats for 128 tokens. Returns rmb [128, 2, 128] fp16:
                [:,0]=rsig broadcast, [:,1]=mu*rsig broadcast."""
                inv = 1.0 / 1024.0
                nc.vector.tensor_tensor(out=sq_t[:], in0=src[:, 0:8, cs],
                                        in1=src[:, 0:8, cs], op=ALU.mult)
                nc.vector.tensor_tensor(out=st1_t[:], in0=src[:, 0:4, cs],
                                        in1=src[:, 4:8, cs], op=ALU.add)
                nc.vector.tensor_tensor(out=su1_t[:], in0=sq_t[:, 0:4, :],
                                        in1=sq_t[:, 4:8, :], op=ALU.add)
                nc.vector.tensor_tensor(out=st2_t[:], in0=st1_t[:, 0:2, :],
                                        in1=st1_t[:, 2:4, :], op=ALU.add)
                nc.vector.tensor_tensor(out=su2_t[:], in0=su1_t[:, 0:2, :],
                                        in1=su1_t[:, 2:4, :], op=ALU.add)
                nc.vector.tensor_tensor(out=stsu_t[:, 0, :],
                                        in0=st2_t[:, 0, :],
                                        in1=st2_t[:, 1, :], op=ALU.add)
                nc.vector.tensor_tensor(out=stsu_t[:, 1, :],
                                        in0=su2_t[:, 0, :],
                                        in1=su2_t[:, 1, :], op=ALU.add)
                ps_s = pst.tile([1, 256], F32, tag="stat")
                nc.tensor.matmul(ps_s[:], ones_t[:, 0:1], stsu_t[:],
                                 start=True, stop=True)
                mu = tps.tile([1, 128], F32, tag="mu")
                nc.vector.tensor_scalar_mul(mu[:], ps_s[:, 0:128], inv)
                ex2 = tps.tile([1, 128], F32, tag="ex2")
                nc.vector.tensor_scalar(out=ex2[:], in0=ps_s[:, 128:256],
                                        scalar1=inv, scalar2=EPS,
                                        op0=ALU.mult, op1=ALU.add)
                mus = tps.tile([1, 128], F32, tag="mus")
                nc.vector.tensor_tensor(out=mus[:], in0=mu[:], in1=mu[:],
                                        op=ALU.mult)
                var = tps.tile([1, 128], F32, tag="var")
                nc.vector.tensor_tensor(out=var[:], in0=ex2[:], in1=mus[:],
                                        op=ALU.subtract)
                vrec = tps.tile([1, 128], F32, tag="vrec")
                nc.vector.reciprocal_approx_fast(out=vrec[:], in_=var[:])
                rsig = tps.tile([1, 128], F32, tag="rsig")
                nc.scalar.activation(rsig[:], vrec[:], AF.Sqrt)
                nc.vector.tensor_copy(rm16_t[0:1, 0, :], rsig[:])
                musig = tps.tile([1, 128], F32, tag="musig")
                nc.vector.tensor_tensor(out=musig[:], in0=mu[:], in1=rsig[:],
                                        op=ALU.mult)
                nc.vector.tensor_copy(rm16_t[0:1, 1, :], musig[:])
                ps_b = pat.tile([128, 512], F32, tag="att")
                nc.tensor.matmul(ps_b[:, 0:256], ones_t[0:1, :],
                                 rm16_t[0:1, :, :], start=True, stop=True)
                rmb = tp.tile([128, 2, 128], FP16, tag="rmb")
                nc.vector.tensor_copy(rmb[:], ps_b[:, 0:256])
                return rmb

            def norm_half(src, dst, cs, rmb, gcol=None, bcol=None):
                bsh = [128, 8, 128]
                nc.vector.tensor_tensor(out=dst[:, :, cs], in0=src[:, :, cs],
                                        in1=rmb[:, 0:1, :].broadcast_to(bsh),
                                        op=ALU.mult)
                nc.vector.tensor_tensor(out=dst[:, :, cs], in0=dst[:, :, cs],
                                        in1=rmb[:, 1:2, :].broadcast_to(bsh),
                                        op=ALU.subtract)
                if gcol is not None:
                    for fc in range(8):
                        nc.vector.tensor_scalar(out=dst[:, fc, cs],
                                                in0=dst[:, fc, cs],
                                                scalar1=gcol[:, fc:fc + 1],
                                                scalar2=bcol[:, fc:fc + 1],
                                                op0=ALU.mult, op1=ALU.add)

            def resid_half(hs, ari):
                hi = 0 if hs.start == 0 else 1
                nc.vector.tensor_tensor(
                    out=h_t[:, :, hs], in0=h_t[:, :, hs],
                    in1=ari[:, hi, :, :], op=ALU.add)

            def gemm_ps(w_t, src, cs, nout, nk, w1row=None):
                """Wide-PSUM gemm over token slice cs (width 128).
                Optional w1row: [1, nout*128] fp16 bias via outer product."""
                ps = pmm.tile([128, 1024], F32, tag="mm")
                for oc in range(nout):
                    for kc in range(nk):
                        last = (kc == nk - 1) and w1row is None
                        nc.tensor.matmul(ps[:, oc * 128:(oc + 1) * 128],
                                         w_t[:, kc, oc * 128:(oc + 1) * 128],
                                         src[:, kc, cs],
                                         start=(kc == 0), stop=last)
                    if w1row is not None:
                        nc.tensor.matmul(ps[:, oc * 128:(oc + 1) * 128],
                                         w1row[0:1, oc * 128:(oc + 1) * 128],
                                         masks_t[0:1, 0:128],
                                         start=False, stop=True)
                return ps

            def launch_ar(tag, src, hs):
                hi = 0 if hs.start == 0 else 1
                bin_ = dp.tile([128, 1024], FP16, tag=f"i{tag}")
                bout = dp.tile([128, 1024], FP16, tag=f"o{tag}")
                nc.sync.dma_start(bin_[:, 0:512], src[:, hi, 0:4, :])
                nc.sync.dma_start(bin_[:, 512:1024], src[:, hi, 4:8, :])
                nc.gpsimd.collective_compute(
                    "AllReduce", ALU.add, replica_groups=RG4,
                    ins=[bin_[:].opt()], outs=[bout[:].opt()])
                return bout

            def land_ar(bout, dst, hs):
                hi = 0 if hs.start == 0 else 1
                nc.sync.dma_start(dst[:, hi, 0:4, :], bout[:, 0:512])
                nc.sync.dma_start(dst[:, hi, 4:8, :], bout[:, 512:1024])

            def attn_front(hs, l, lb_t, wqkv_t):
                """resid + LN + qkv for one stream."""
                if l > 0:
                    resid_half(hs, ari2_t)
                rmb = stats_half(h_t, hs)
                norm_half(h_t, n_t, hs, rmb)
                ps = gemm_ps(wqkv_t, n_t, hs, 6, 8)
                nc.vector.tensor_tensor(out=qkv_t[:, :, hs], in0=ps[:, 0:768],
                                        in1=lb_t[:, 0:6, :], op=ALU.add)

            def attn_A():
                """Heads attend within tokens 0:128 (kt=0 block only)."""
                nc.sync.dma_start(qkvh2_t[:, :, T0], qkv_t[64:128, :, T0])
                for hh in range(4):
                    rq = 64 * (hh % 2)
                    src = qkv_t if rq == 0 else qkvh2_t
                    qc, kc2, vc = hh // 2, 2 + hh // 2, 4 + hh // 2
                    ps_s = pat.tile([128, 512], F32, tag="att")
                    nc.tensor.matmul(ps_s[:, 0:128],
                                     src[0:64, kc2, 0:128],
                                     src[0:64, qc, 0:128],
                                     start=True, stop=True)
                    em = tp.tile([128, 512], FP16, tag="em")
                    nc.scalar.activation(em[:, 0:128], ps_s[:, 0:128],
                                         AF.Exp, scale=0.125)
                    nc.vector.tensor_tensor(out=em[:, 0:128], in0=em[:, 0:128],
                                            in1=masks_t[:, 0:128],
                                            op=ALU.mult)
                    ps_d = pst.tile([1, 256], F32, tag="stat")
                    nc.tensor.matmul(ps_d[:, 0:128], ones_t[:, 0:1],
                                     em[:, 0:128], start=True, stop=True)
                    rec = tps.tile([1, 256], F32, tag="rec")
                    nc.vector.reciprocal_approx_fast(out=rec[:, 0:128],
                                                     in_=ps_d[:, 0:128])
                    rec16 = tps.tile([1, 256], FP16, tag="rec16")
                    nc.vector.tensor_copy(rec16[:, 0:128], rec[:, 0:128])
                    ps_rb = pat.tile([128, 512], F32, tag="att")
                    nc.tensor.matmul(ps_rb[:, 0:128], ones_t[0:1, :],
                                     rec16[:, 0:128], start=True, stop=True)
                    rec_b = tp.tile([128, 256], FP16, tag="recb")
                    nc.vector.tensor_copy(rec_b[:, 0:128], ps_rb[:, 0:128])
                    ps_t = ptr.tile([128, 128], FP16, tag="ptr")
                    nc.tensor.transpose(ps_t[:, 0:64],
                                        src[0:64, vc, 0:128],
                                        ident_t[0:64, 0:64])
                    nc.vector.tensor_copy(vtok_t[:, hh, 0, :], ps_t[:, 0:64])
                    ps_o = pat.tile([128, 512], F32, tag="att")
                    nc.tensor.matmul(ps_o[0:64, 0:128], vtok_t[:, hh, 0, :],
                                     em[:, 0:128], start=True, stop=True)
                    if rq == 0:
                        nc.vector.tensor_tensor(out=A_t[0:64, hh // 2, T0],
                                                in0=ps_o[0:64, 0:128],
                                                in1=rec_b[0:64, 0:128],
                                                op=ALU.mult)
                    else:
                        oh = tp.tile([64, 256], FP16, tag="oh")
                        nc.vector.tensor_tensor(out=oh[:, 0:128],
                                                in0=ps_o[0:64, 0:128],
                                                in1=rec_b[0:64, 0:128],
                                                op=ALU.mult)
                        nc.sync.dma_start(A_t[64:128, hh // 2, T0],
                                          oh[:, 0:128])

            def attn_B():
                """Queries 128:256 attend to keys 0:256 (mask on kt=1)."""
                nc.sync.dma_start(qkvh2_t[:, :, T1], qkv_t[64:128, :, T1])
                for hh in range(4):
                    rq = 64 * (hh % 2)
                    src = qkv_t if rq == 0 else qkvh2_t
                    qc, kc2, vc = hh // 2, 2 + hh // 2, 4 + hh // 2
                    ps_s = pat.tile([128, 512], F32, tag="att")
                    for kt in range(2):
                        nc.tensor.matmul(
                            ps_s[:, kt * 128:(kt + 1) * 128],
                            src[0:64, kc2, kt * 128:(kt + 1) * 128],
                            src[0:64, qc, T1], start=True, stop=True)
                    em = tp.tile([128, 512], FP16, tag="em")
                    nc.scalar.activation(em[:, 0:256], ps_s[:, 0:256],
                                         AF.Exp, scale=0.125)
                    nc.vector.tensor_tensor(out=em[:, 128:256],
                                            in0=em[:, 128:256],
                                            in1=masks_t[:, 384:512],
                                            op=ALU.mult)
                    ps_d = pst.tile([1, 256], F32, tag="stat")
                    for kt in range(2):
                        nc.tensor.matmul(ps_d[:, 0:128], ones_t[:, 0:1],
                                         em[:, kt * 128:(kt + 1) * 128],
                                         start=(kt == 0), stop=(kt == 1))
                    rec = tps.tile([1, 256], F32, tag="rec")
                    nc.vector.reciprocal_approx_fast(out=rec[:, 0:128],
                                                     in_=ps_d[:, 0:128])
                    rec16 = tps.tile([1, 256], FP16, tag="rec16")
                    nc.vector.tensor_copy(rec16[:, 0:128], rec[:, 0:128])
                    ps_rb = pat.tile([128, 512], F32, tag="att")
                    nc.tensor.matmul(ps_rb[:, 0:128], ones_t[0:1, :],
                                     rec16[:, 0:128], start=True, stop=True)
                    rec_b = tp.tile([128, 256], FP16, tag="recb")
                    nc.vector.tensor_copy(rec_b[:, 0:128], ps_rb[:, 0:128])
                    ps_t = ptr.tile([128, 128], FP16, tag="ptr")
                    nc.tensor.transpose(ps_t[:, 0:64],
                                        src[0:64, vc, T1],
                                        ident_t[0:64, 0:64])
                    nc.vector.tensor_copy(vtok_t[:, hh, 1, :], ps_t[:, 0:64])
                    ps_o = pat.tile([128, 512], F32, tag="att")
                    for kt in range(2):
                        nc.tensor.matmul(ps_o[0:64, 0:128],
                                         vtok_t[:, hh, kt, :],
                                         em[:, kt * 128:(kt + 1) * 128],
                                         start=(kt == 0), stop=(kt == 1))
                    if rq == 0:
                        nc.vector.tensor_tensor(out=A_t[0:64, hh // 2, T1],
                                                in0=ps_o[0:64, 0:128],
                                                in1=rec_b[0:64, 0:128],
                                                op=ALU.mult)
                    else:
                        oh = tp.tile([64, 256], FP16, tag="oh")
                        nc.vector.tensor_tensor(out=oh[:, 0:128],
                                                in0=ps_o[0:64, 0:128],
                                                in1=rec_b[0:64, 0:128],
                                                op=ALU.mult)
                        nc.sync.dma_start(A_t[64:128, hh // 2, T1],
                                          oh[:, 0:128])

            def wo_ar(hs, l, lb_t, wo_t):
                hi = 0 if hs.start == 0 else 1
                ps = gemm_ps(wo_t, A_t, hs, 8, 2)
                nc.vector.tensor_tensor(out=aro_t[:, hi, :, :], in0=ps[:],
                                        in1=lb_t[:, 6:14, :], op=ALU.add)
                return launch_ar(f"a{l}h{hi}", aro_t, hs)

            def mlp(hs, l, lb_t, w1_t, w2_t, w1r_t, ar_a):
                hi = 0 if hs.start == 0 else 1
                land_ar(ar_a, ari_t, hs)
                resid_half(hs, ari_t)
                rmb = stats_half(h_t, hs)
                norm_half(h_t, n_t, hs, rmb)
                ps = gemm_ps(w1_t, n_t, hs, 8, 8, w1row=w1r_t)
                nc.scalar.activation(gu_t[:, :, hs], ps[:], AF.Gelu)
                ps2 = gemm_ps(w2_t, gu_t, hs, 8, 8)
                nc.vector.tensor_tensor(out=aro2_t[:, hi, :, :], in0=ps2[:],
                                        in1=lb_t[:, 14:22, :], op=ALU.add)
                return launch_ar(f"m{l}h{hi}", aro2_t, hs)

            # ---------------- patch pooling: h = patchesT ----------------
            for hs in (T0, T1):
                ps = pmm.tile([128, 1024], F32, tag="mm")
                for oc in range(8):
                    for kc in range(2):
                        nc.tensor.matmul(
                            ps[:, oc * 128:(oc + 1) * 128],
                            embS_t[:, kc, oc * 128:(oc + 1) * 128],
                            cnt_t[:, kc, hs],
                            start=(kc == 0), stop=(kc == 1))
                nc.vector.tensor_copy(h_t[:, :, hs], ps[:])

            # ---------------- transformer layers ----------------
            ar_m = [None, None]
            for l in range(L):
                wqkv_t = wp.tile([128, 8, 768], FP16, tag="wqkv", bufs=1)
                for q in range(2):
                    nc.sync.dma_start(
                        wqkv_t[:, q * 4:(q + 1) * 4, :],
                        d["wqkv"][l].rearrange("p (kc x) -> p kc x", kc=8)
                        [:, q * 4:(q + 1) * 4, :])
                wo_t = wp.tile([128, 2, 1024], FP16, tag="wo", bufs=1)
                nc.sync.dma_start(wo_t[:], d["wo"][l].rearrange(
                    "p (kc x) -> p kc x", kc=2))
                w1_t = wp.tile([128, 8, 1024], FP16, tag="w1")
                for q in range(2):
                    nc.sync.dma_start(
                        w1_t[:, q * 4:(q + 1) * 4, :],
                        d["w1"][l].rearrange("p (kc x) -> p kc x", kc=8)
                        [:, q * 4:(q + 1) * 4, :])
                w2_t = wp.tile([128, 8, 1024], FP16, tag="w2")
                for q in range(2):
                    nc.sync.dma_start(
                        w2_t[:, q * 4:(q + 1) * 4, :],
                        d["w2"][l].rearrange("p (kc x) -> p kc x", kc=8)
                        [:, q * 4:(q + 1) * 4, :])
                lb_t = wp.tile([128, 22, 128], FP16, tag="lb", bufs=1)
                nc.sync.dma_start(lb_t[:], d["lb"][l].rearrange(
                    "p (kc x) -> p kc x", kc=22))
                w1r_t = wp.tile([1, 1024], FP16, tag="w1r", bufs=1)
                nc.sync.dma_start(w1r_t[:], d["w1r"][l])

                # stream A attention sublayer, AR flies under stream B front
                attn_front(T0, l, lb_t, wqkv_t)
                attn_A()
                ar_a0 = wo_ar(T0, l, lb_t, wo_t)

                # stream B attention sublayer
                attn_front(T1, l, lb_t, wqkv_t)
                attn_B()
                ar_a1 = wo_ar(T1, l, lb_t, wo_t)

                # byte-path filler (independent of ARs)
                if l == 0:
                    for hs in (T0, T1):
                        rmb = stats_half(embT_t, hs)
                        norm_half(embT_t, qn_t, hs, rmb)
                elif l == 1:
                    ps = pat.tile([128, 512], F32, tag="att")
                    for oc in range(2):
                        for kc in range(8):
                            nc.tensor.matmul(
                                ps[:, oc * 256:(oc + 1) * 256],
                                wq_t[:, kc, oc * 128:(oc + 1) * 128],
                                qn_t[:, kc, :],
                                start=(kc == 0), stop=(kc == 7))
                    nc.vector.tensor_tensor(out=qT_t[:], in0=ps[:],
                                            in1=cabR_t[:, 0:2, :],
                                            op=ALU.add)
                elif l == 2:
                    ps = pat.tile([128, 512], F32, tag="att")
                    for oc in range(2):
                        for kc in range(8):
                            nc.tensor.matmul(
                                ps[:, oc * 256:(oc + 1) * 256],
                                headw_t[:, kc, oc * 128:(oc + 1) * 128],
                                embT_t[:, kc, :],
                                start=(kc == 0), stop=(kc == 7))
                    nc.vector.tensor_copy(et_t[:], ps[:])
                else:
                    ps = pat.tile([128, 512], F32, tag="att")
                    for oc in range(2):
                        for kc in range(8):
                            nc.tensor.matmul(
                                ps[:, oc * 256:(oc + 1) * 256],
                                cawoT_t[:, kc, oc * 128:(oc + 1) * 128],
                                headw_t[:, kc, :],
                                start=(kc == 0), stop=(kc == 7))
                    nc.vector.tensor_copy(w2c_t[:], ps[:])

                # mlp sublayers: A then B; each AR hides under the next block
                ar_m[0] = mlp(T0, l, lb_t, w1_t, w2_t, w1r_t, ar_a0)
                ar_m[1] = mlp(T1, l, lb_t, w1_t, w2_t, w1r_t, ar_a1)
                for hi, hs in enumerate((T0, T1)):
                    land_ar(ar_m[hi], ari2_t, hs)

            # ---------------- final: resid + fn/ca norm -> kvn ----------
            kvn_t = n_t
            if skip_kvn_ln:
                for hs in (T0, T1):
                    resid_half(hs, ari2_t)
                    rmb = stats_half(h_t, hs)
                    norm_half(h_t, kvn_t, hs, rmb)
            else:
                pf_t = gu_t
                for hs in (T0, T1):
                    resid_half(hs, ari2_t)
                    rmb = stats_half(h_t, hs)
                    norm_half(h_t, pf_t, hs, rmb,
                              gcol=cbias_t[:, 2:10], bcol=cbias_t[:, 10:18])
                for hs in (T0, T1):
                    rmb = stats_half(pf_t, hs)
                    norm_half(pf_t, kvn_t, hs, rmb)

            # ---------------- CA k/v projections ----------------
            for (w_t, out_t, bc0) in ((wk_t, kT_t, 2), (wv_t, vT_t, 4)):
                ps = pat.tile([128, 512], F32, tag="att")
                for oc in range(2):
                    for kc in range(8):
                        nc.tensor.matmul(
                            ps[:, oc * 256:(oc + 1) * 256],
                            w_t[:, kc, oc * 128:(oc + 1) * 128],
                            kvn_t[:, kc, :], start=(kc == 0), stop=(kc == 7))
                nc.vector.tensor_tensor(out=out_t[:], in0=ps[:],
                                        in1=cabR_t[:, bc0:bc0 + 2, :],
                                        op=ALU.add)

            # ---------------- CA attention (2 heads, dh=128) ----------
            for chh in range(2):
                ps_s = pat.tile([128, 512], F32, tag="att")
                for kt in range(2):
                    nc.tensor.matmul(
                        ps_s[:, kt * 256:(kt + 1) * 256],
                        kT_t[:, chh, kt * 128:(kt + 1) * 128],
                        qT_t[:, chh, :], start=True, stop=True)
                em = tp.tile([128, 512], FP16, tag="em")
                nc.scalar.activation(em[:], ps_s[:], AF.Exp,
                                     scale=float(1.0 / np.sqrt(128.0)))
                ps_d = pst.tile([1, 256], F32, tag="stat")
                for kt in range(2):
                    nc.tensor.matmul(ps_d[:], ones_t[:, 0:1],
                                     em[:, kt * 256:(kt + 1) * 256],
                                     start=(kt == 0), stop=(kt == 1))
                rec = tps.tile([1, 256], F32, tag="rec")
                nc.vector.reciprocal_approx_fast(out=rec[:], in_=ps_d[:])
                rec16 = tps.tile([1, 256], FP16, tag="rec16")
                nc.vector.tensor_copy(rec16[:], rec[:])
                ps_rb = pat.tile([128, 512], F32, tag="att")
                nc.tensor.matmul(ps_rb[:, 0:256], ones_t[0:1, :], rec16[:],
                                 start=True, stop=True)
                rec_b = tp.tile([128, 256], FP16, tag="recb")
                nc.vector.tensor_copy(rec_b[:], ps_rb[:, 0:256])
                vtokca = tp.tile([128, 2, 128], FP16, tag="vtokca")
                for kt in range(2):
                    ps_t = ptr.tile([128, 128], FP16, tag="ptr")
                    nc.tensor.transpose(
                        ps_t[:], vT_t[:, chh, kt * 128:(kt + 1) * 128],
                        ident_t[:])
                    nc.vector.tensor_copy(vtokca[:, kt, :], ps_t[:])
                ps_o = pat.tile([128, 512], F32, tag="att")
                for kt in range(2):
                    nc.tensor.matmul(ps_o[:, 0:256], vtokca[:, kt, :],
                                     em[:, kt * 256:(kt + 1) * 256],
                                     start=(kt == 0), stop=(kt == 1))
                nc.vector.tensor_tensor(out=O_t[:, chh, :],
                                        in0=ps_o[:, 0:256],
                                        in1=rec_b[:], op=ALU.mult)

            # ---------------- logits partials + AR (fp16) ----------------
            lp_t = sbp.tile([128, 2, 256], FP16, tag="lp")
            ps = pat.tile([128, 512], F32, tag="att")
            for vo in range(2):
                for od in range(2):
                    nc.tensor.matmul(ps[:, vo * 256:(vo + 1) * 256],
                                     w2c_t[:, od, vo * 128:(vo + 1) * 128],
                                     O_t[:, od, :],
                                     start=(od == 0), stop=(od == 1))
            nc.vector.tensor_copy(lp_t[:], ps[:])
            lbin = dp.tile([128, 512], FP16, tag="lci")
            lbout = dp.tile([128, 512], FP16, tag="lco")
            nc.sync.dma_start(lbin[:], lp_t[:])
            nc.gpsimd.collective_compute(
                "AllReduce", ALU.add, replica_groups=RG4,
                ins=[lbin[:].opt()], outs=[lbout[:].opt()])
            lar_t = sbp.tile([128, 2, 256], FP16, tag="lar")
            nc.sync.dma_start(lar_t[:], lbout[:])

            out_t = sbp.tile([128, 2, 256], F32, tag="outt")
            for vo in range(2):
                nc.vector.tensor_scalar(out=out_t[:, vo, :],
                                        in0=lar_t[:, vo, :],
                                        scalar1=cbias_t[:, vo:vo + 1],
                                        scalar2=None, op0=ALU.add)
                nc.vector.tensor_tensor(out=out_t[:, vo, :],
                                        in0=out_t[:, vo, :],
                                        in1=et_t[:, vo, :], op=ALU.add)
            nc.sync.dma_start(out_d[:], out_t[:])

    nc.compile()
    nc.m = get_hw_module(nc.m)
    return nc


# --------------------------------------------------------------------------
# host side
# --------------------------------------------------------------------------
def _shuf16(M):
    """[K, X] -> [128, (K//128)*X] fp16 laid out as [p, kc, x]."""
    K, X = M.shape
    return np.ascontiguousarray(
        M.reshape(K // 128, 128, X).transpose(1, 0, 2).reshape(128, -1)
    ).astype(np.float16)


def _rep(bias, nc_, w):
    """bias [nc_*128] -> [128, nc_, w] fp16 replicated along tokens."""
    return np.broadcast_to(
        bias.reshape(nc_, 128).T[:, :, None], (128, nc_, w)
    ).astype(np.float16)


def _prep(inputs):
    f = lambda k: np.asarray(inputs[k], np.float32)
    byte_seq = np.asarray(inputs["byte_seq"])
    bd = np.asarray(inputs["patch_boundaries"])
    emb = f("emb")

    pos = np.arange(S)
    pid = np.stack([np.searchsorted(bd[b], pos, side="right")
                    for b in range(B)])
    pid = np.clip(pid, 0, P - 1)
    Cn = np.zeros((B, P, V), np.float32)
    for b in range(B):
        np.add.at(Cn[b], (pid[b], byte_seq[b]), 1.0)
    cnts = Cn.sum(-1)
    Cn /= np.maximum(cnts, 1.0)[..., None]

    g1, b1a = f("g_ln1_g"), f("g_ln1_b")
    g2, b2a = f("g_ln2_g"), f("g_ln2_b")
    Wqkv, bqkv = f("g_wqkv"), f("g_bqkv")
    Wo, bo = f("g_wo"), f("g_bo")
    W1, b1 = f("g_w1"), f("g_b1")
    W2, b2 = f("g_w2"), f("g_b2")

    Wq_f = g1[:, :, None] * Wqkv
    biasq = np.einsum("lh,lho->lo", b1a, Wqkv) + bqkv
    W1_f = g2[:, :, None] * W1
    bias1 = np.einsum("lh,lho->lo", b2a, W1) + b1

    ca_wqkv, ca_bqkv = f("ca_wqkv"), f("ca_bqkv")
    ca_wo, ca_bo = f("ca_wo"), f("ca_bo")
    head_w, head_b = f("head_w"), f("head_b")
    cag, cab = f("ca_ln_g"), f("ca_ln_b")
    headb_full = head_b + ca_bo @ head_w

    wq_e = cag[:, None] * ca_wqkv[:, :H]
    bq_e = cab @ ca_wqkv[:, :H] + ca_bqkv[:H]
    wk_e = cag[:, None] * ca_wqkv[:, H:2 * H]
    bk_e = cab @ ca_wqkv[:, H:2 * H] + ca_bqkv[H:2 * H]
    wv_e = cag[:, None] * ca_wqkv[:, 2 * H:]
    bv_e = cab @ ca_wqkv[:, 2 * H:] + ca_bqkv[2 * H:]

    masks = np.zeros((128, 2, 256), np.float32)
    for kt in range(2):
        ktg = kt * 128 + np.arange(128)
        masks[:, kt, :] = (ktg[:, None] <= np.arange(256)[None, :])

    in_maps = []
    for c in range(NC):
        b, r = c // 4, c % 4
        m = {}
        cols = np.concatenate([np.arange(r * 256, (r + 1) * 256) + k * H
                               for k in range(3)])
        m["wqkv"] = np.stack([_shuf16(Wq_f[l][:, cols]) for l in range(L)])
        m["wo"] = np.stack([_shuf16(Wo[l][r * 256:(r + 1) * 256, :])
                            for l in range(L)])
        m["w1"] = np.stack(
            [_shuf16(W1_f[l][:, r * 1024:(r + 1) * 1024]) for l in range(L)])
        m["w2"] = np.stack(
            [_shuf16(W2[l][r * 1024:(r + 1) * 1024, :]) for l in range(L)])
        lb = np.zeros((L, 128, 22, 128), np.float16)
        for l in range(L):
            lb[l, :, 0:6, :] = _rep(biasq[l, cols], 6, 128)
            lb[l, :, 6:14, :] = _rep(bo[l] / 4, 8, 128)
            lb[l, :, 14:22, :] = _rep(b2[l] / 4, 8, 128)
        m["lb"] = np.ascontiguousarray(lb.reshape(L, 128, 2816))
        m["w1r"] = np.ascontiguousarray(
            bias1[:, r * 1024:(r + 1) * 1024][:, None, :]).astype(np.float16)
        cabR = np.concatenate([
            _rep(bq_e[r * 256:(r + 1) * 256], 2, 256),
            _rep(bk_e[r * 256:(r + 1) * 256], 2, 256),
            _rep(bv_e[r * 256:(r + 1) * 256], 2, 256)], axis=1)
        m["cabR"] = np.ascontiguousarray(cabR.reshape(128, 1536))
        m["cnt"] = _shuf16(Cn[b].T)
        m["embS"] = _shuf16(emb)
        m["embT"] = _shuf16(np.ascontiguousarray(emb.T))
        m["masks"] = np.ascontiguousarray(
            masks.reshape(128, 512)).astype(np.float16)
        m["ones"] = np.ones((128, 128), np.float16)
        m["ident"] = np.eye(128, dtype=np.float16)
        m["wq"] = _shuf16(wq_e[:, r * 256:(r + 1) * 256])
        m["wk"] = _shuf16(wk_e[:, r * 256:(r + 1) * 256])
        m["wv"] = _shuf16(wv_e[:, r * 256:(r + 1) * 256])
        m["cawoT"] = _shuf16(np.ascontiguousarray(
            ca_wo[r * 256:(r + 1) * 256, :].T))
        m["headw"] = _shuf16(head_w)
        cbias = np.zeros((128, 18), np.float32)
        cbias[:, 0:2] = headb_full.reshape(2, 128).T
        cbias[:, 2:10] = f("fn_g").reshape(8, 128).T
        cbias[:, 10:18] = f("fn_b").reshape(8, 128).T
        m["cbias"] = np.ascontiguousarray(cbias)
        in_maps.append(m)
    return in_maps, byte_seq


def run_device(inputs, trace=False):
    skip = (np.allclose(np.asarray(inputs["fn_g"]), 1.0)
            and np.allclose(np.asarray(inputs["fn_b"]), 0.0))
    key = ("nc", skip)
    if key not in _CACHE:
        _CACHE[key] = _trace(skip)
    nc = _CACHE[key]
    in_maps, byte_seq = _prep(inputs)
    res = run_bass_kernel_spmd(nc, in_maps, core_ids=list(range(NC)),
                               trace=trace)
    out = np.empty((B, S, V), np.float32)
    for b in range(B):
        ltab = res.results[b * 4]["ltab"]
        tab = ltab.reshape(128, 2, 256).transpose(1, 0, 2).reshape(256, 256)
        out[b] = tab.T[byte_seq[b]]
    return out, res


def kernel(**inputs) -> np.ndarray:
    out, _ = run_device(inputs, trace=False)
    return out


# revision 3
# speedup vs baseline: 1.0881x; 1.0881x over previous
"""BLT model TRN2 kernel v4 — nn_BLTModel_13872744366807.

v4 over v3: LayerNorm commuted through the qkv/w1 GEMMs — W@h starts right
after the residual lands; the -wsum*mu term accumulates as a 1-partition
outer-product matmul and *rsig applies in the drain, so LN stats never gate
the PE. 8-way weight DMA splits. v3 notes:
- Two-stream software pipeline: stream A = patch tokens 0:128, stream B =
  128:256. Causal attention makes A fully independent of B (B reads A's
  K/V). Each AllReduce flies under the other stream's sublayer compute.
- Biases via host-replicated fp16 tiles fused into the PSUM drain add
  (v2's outer-product bias matmuls cost 60+us of PE).
- fp16 logits AllReduce.
Everything else as v2: DP-2 x TP-4, vocab collapse, fp16 compute with
fp32 PSUM, direct LN with host-folded affines, wide-PSUM GEMMs.
"""
import numpy as np
import concourse.bacc as bacc
import concourse.bass as bass
import concourse.mybir as mybir
from concourse import tile
from concourse.bass_utils import run_bass_kernel_spmd
from concourse.bass_interp import get_hw_module

F32 = mybir.dt.float32
FP16 = mybir.dt.float16
AF = mybir.ActivationFunctionType
ALU = mybir.AluOpType

L, B, S, P, H, V, NC = 4, 2, 4096, 256, 1024, 256, 8
EPS = 1e-6
RG4 = [[0, 1, 2, 3], [4, 5, 6, 7]]
T0 = slice(0, 128)
T1 = slice(128, 256)

_CACHE = {}


def _trace(skip_kvn_ln):
    nc = bacc.Bacc("TRN2", target_bir_lowering=False, debug=False,
                   num_devices=NC)
    d = {}

    def inp(name, shape, dt=FP16):
        d[name] = nc.dram_tensor(name, shape, dt, kind="ExternalInput").ap()

    inp("wqkv", [L, 128, 6144])
    inp("wo", [L, 128, 2048])
    inp("w1", [L, 128, 8192])
    inp("w2", [L, 128, 8192])
    # replicated fp16 bias tiles: [0:768) qkv, [768:1792) wo,
    #   [1792:2816) w2, [2816:3840) w1 (pre-Gelu)
    inp("lb", [L, 128, 3840])
    # negated LN-commute colsum rows: [0:768) -wsum_qkv, [768:1792) -wsum_w1
    inp("wsrow", [L, 1, 1792])
    # negated colsums for CA gemms: [0:256) wq, [256:512) wk, [512:768) wv
    inp("cawsrow", [1, 768])
    # replicated CA q/k/v biases: [p, {q0,q1,k0,k1,v0,v1}, 256]
    inp("cabR", [128, 1536])
    inp("cnt", [128, 512])
    inp("embS", [128, 2048])
    inp("embT", [128, 2048])
    inp("masks", [128, 512])
    inp("ones", [128, 128])
    inp("ident", [128, 128])
    inp("wq", [128, 2048]); inp("wk", [128, 2048]); inp("wv", [128, 2048])
    inp("cawoT", [128, 2048])
    inp("headw", [128, 2048])
    # f32 cols: [0:2) headb, [2:10) fng, [10:18) fnb
    inp("cbias", [128, 18], F32)
    out_d = nc.dram_tensor("ltab", [128, 512], F32, kind="ExternalOutput").ap()

    with tile.TileContext(nc) as tc:
        with (
            tc.tile_pool(name="const", bufs=1) as cp,
            tc.tile_pool(name="sb", bufs=1) as sbp,
            tc.tile_pool(name="wts", bufs=2) as wp,
            tc.tile_pool(name="tmp", bufs=2) as tp,
            tc.tile_pool(name="tps", bufs=1) as tps,
            tc.tile_pool(name="mm", bufs=2, space="PSUM") as pmm,
            tc.tile_pool(name="att", bufs=2, space="PSUM") as pat,
            tc.tile_pool(name="ptr", bufs=1, space="PSUM") as ptr,
            tc.tile_pool(name="st", bufs=1, space="PSUM") as pst,
            tc.tile_pool(name="dram", bufs=1, space="DRAM") as dp,
        ):
            # ---------------- constants ----------------
            def cload(name, shape, dt=FP16):
                t_ = cp.tile(shape, dt, tag=name)
                nc.sync.dma_start(t_[:], d[name][:])
                return t_

            embS_t = cp.tile([128, 2, 1024], FP16, tag="embS")
            nc.sync.dma_start(embS_t[:], d["embS"][:].rearrange(
                "p (kc x) -> p kc x", kc=2))
            cnt_t = cp.tile([128, 2, 256], FP16, tag="cnt")
            nc.sync.dma_start(cnt_t[:], d["cnt"][:].rearrange(
                "p (kc x) -> p kc x", kc=2))
            ones_t = cload("ones", [128, 128])
            ident_t = cload("ident", [128, 128])
            masks_t = cload("masks", [128, 512])
            cbias_t = cload("cbias", [128, 18], F32)
            cawsrow_t = cp.tile([1, 768], FP16, tag="cawsrow")
            nc.sync.dma_start(cawsrow_t[:], d["cawsrow"][:])
            cabR_t = cp.tile([128, 6, 256], FP16, tag="cabR")
            nc.sync.dma_start(cabR_t[:], d["cabR"][:].rearrange(
                "p (kc x) -> p kc x", kc=6))
            embT_t = cp.tile([128, 8, 256], FP16, tag="embT")
            nc.sync.dma_start(embT_t[:], d["embT"][:].rearrange(
                "p (kc x) -> p kc x", kc=8))
            wq_t = cp.tile([128, 8, 256], FP16, tag="wq")
            nc.sync.dma_start(wq_t[:], d["wq"][:].rearrange(
                "p (kc x) -> p kc x", kc=8))
            wk_t = cp.tile([128, 8, 256], FP16, tag="wk")
            nc.sync.dma_start(wk_t[:], d["wk"][:].rearrange(
                "p (kc x) -> p kc x", kc=8))
            wv_t = cp.tile([128, 8, 256], FP16, tag="wv")
            nc.sync.dma_start(wv_t[:], d["wv"][:].rearrange(
                "p (kc x) -> p kc x", kc=8))
            cawoT_t = cp.tile([128, 8, 256], FP16, tag="cawoT")
            nc.sync.dma_start(cawoT_t[:], d["cawoT"][:].rearrange(
                "p (kc x) -> p kc x", kc=8))
            headw_t = cp.tile([128, 8, 256], FP16, tag="headw")
            nc.sync.dma_start(headw_t[:], d["headw"][:].rearrange(
                "p (kc x) -> p kc x", kc=8))

            # cc warm-up: tiny AllReduce early
            wbin = dp.tile([128, 8], F32, tag="wrmi")
            wbout = dp.tile([128, 8], F32, tag="wrmo")
            nc.sync.dma_start(wbin[:], d["cbias"][:, 0:8])
            nc.gpsimd.collective_compute(
                "AllReduce", ALU.add, replica_groups=RG4,
                ins=[wbin[:].opt()], outs=[wbout[:].opt()])

            # ---------------- persistent activations ----------------
            h_t = sbp.tile([128, 8, 256], FP16, tag="h")
            sq_t = sbp.tile([128, 8, 128], FP16, tag="sq")
            st1_t = sbp.tile([128, 4, 128], FP16, tag="st1")
            st2_t = sbp.tile([128, 2, 128], FP16, tag="st2")
            su1_t = sbp.tile([128, 4, 128], FP16, tag="su1")
            su2_t = sbp.tile([128, 2, 128], FP16, tag="su2")
            stsu_t = sbp.tile([128, 2, 128], FP16, tag="stsu")
            rm16_t = sbp.tile([1, 2, 128], FP16, tag="rm16")
            qkv_t = sbp.tile([128, 6, 256], FP16, tag="qkv")
            qkvh2_t = sbp.tile([64, 6, 256], FP16, tag="qkvh2")
            vtok_t = sbp.tile([128, 4, 2, 64], FP16, tag="vtok")
            A_t = sbp.tile([128, 2, 256], FP16, tag="A")
            gu_t = sbp.tile([128, 8, 256], FP16, tag="gu")
            aro_t = sbp.tile([128, 2, 8, 128], FP16, tag="aro")
            ari_t = sbp.tile([128, 2, 8, 128], FP16, tag="ari")
            aro2_t = sbp.tile([128, 2, 8, 128], FP16, tag="aro2")
            ari2_t = sbp.tile([128, 2, 8, 128], FP16, tag="ari2")
            qnr_t = sbp.tile([128, 256], FP16, tag="qnr")
            qnmu_t = sbp.tile([1, 256], FP16, tag="qnmu")
            ffr_t = sbp.tile([128, 256], FP16, tag="ffr")
            fmu_t = sbp.tile([1, 256], FP16, tag="fmu")
            qT_t = sbp.tile([128, 2, 256], FP16, tag="qT")
            kT_t = sbp.tile([128, 2, 256], FP16, tag="kT")
            vT_t = sbp.tile([128, 2, 256], FP16, tag="vT")
            O_t = sbp.tile([128, 2, 256], FP16, tag="O")
            w2c_t = sbp.tile([128, 2, 256], FP16, tag="w2c")
            et_t = sbp.tile([128, 2, 256], F32, tag="et")

            # ---------------- helpers ----------------
            def stats_half(src, cs):
                """LN stats for 128 tokens. Returns rmb [128, 2, 128] fp16:
                [:,0]=rsig broadcast, [:,1]=mu*rsig broadcast."""
                inv = 1.0 / 1024.0
                nc.vector.tensor_tensor(out=sq_t[:], in0=src[:, 0:8, cs],
                                        in1=src[:, 0:8, cs], op=ALU.mult)
                nc.vector.tensor_tensor(out=st1_t[:], in0=src[:, 0:4, cs],
                                        in1=src[:, 4:8, cs], op=ALU.add)
                nc.vector.tensor_tensor(out=su1_t[:], in0=sq_t[:, 0:4, :],
                                        in1=sq_t[:, 4:8, :], op=ALU.add)
                nc.vector.tensor_tensor(out=st2_t[:], in0=st1_t[:, 0:2, :],
                                        in1=st1_t[:, 2:4, :], op=ALU.add)
                nc.vector.tensor_tensor(out=su2_t[:], in0=su1_t[:, 0:2, :],
                                        in1=su1_t[:, 2:4, :], op=ALU.add)
                nc.vector.tensor_tensor(out=stsu_t[:, 0, :],
                                        in0=st2_t[:, 0, :],
                                        in1=st2_t[:, 1, :], op=ALU.add)
                nc.vector.tensor_tensor(out=stsu_t[:, 1, :],
                                        in0=su2_t[:, 0, :],
                                        in1=su2_t[:, 1, :], op=ALU.add)
                ps_s = pst.tile([1, 256], F32, tag="stat")
                nc.tensor.matmul(ps_s[:], ones_t[:, 0:1], stsu_t[:],
                                 start=True, stop=True)
                mu = tps.tile([1, 128], F32, tag="mu")
                nc.vector.tensor_scalar_mul(mu[:], ps_s[:, 0:128], inv)
                ex2 = tps.tile([1, 128], F32, tag="ex2")
                nc.vector.tensor_scalar(out=ex2[:], in0=ps_s[:, 128:256],
                                        scalar1=inv, scalar2=EPS,
                                        op0=ALU.mult, op1=ALU.add)
                mus = tps.tile([1, 128], F32, tag="mus")
                nc.vector.tensor_tensor(out=mus[:], in0=mu[:], in1=mu[:],
                                        op=ALU.mult)
                var = tps.tile([1, 128], F32, tag="var")
                nc.vector.tensor_tensor(out=var[:], in0=ex2[:], in1=mus[:],
                                        op=ALU.subtract)
                vrec = tps.tile([1, 128], F32, tag="vrec")
                nc.vector.reciprocal_approx_fast(out=vrec[:], in_=var[:])
                rsig = tps.tile([1, 128], F32, tag="rsig")
                nc.scalar.activation(rsig[:], vrec[:], AF.Sqrt)
                nc.vector.tensor_copy(rm16_t[0:1, 0, :], rsig[:])
                nc.vector.tensor_copy(rm16_t[0:1, 1, :], mu[:])
                ps_b = pat.tile([128, 512], F32, tag="att")
                nc.tensor.matmul(ps_b[:, 0:256], ones_t[0:1, :],
                                 rm16_t[0:1, :, :], start=True, stop=True)
                rmb = tp.tile([128, 2, 128], FP16, tag="rmb")
                nc.vector.tensor_copy(rmb[:], ps_b[:, 0:256])
                return rmb

            def norm_half(src, dst, cs, rmb, gcol=None, bcol=None):
                bsh = [128, 8, 128]
                nc.vector.tensor_tensor(out=dst[:, :, cs], in0=src[:, :, cs],
                                        in1=rmb[:, 1:2, :].broadcast_to(bsh),
                                        op=ALU.subtract)
                nc.vector.tensor_tensor(out=dst[:, :, cs], in0=dst[:, :, cs],
                                        in1=rmb[:, 0:1, :].broadcast_to(bsh),
                                        op=ALU.mult)
                if gcol is not None:
                    for fc in range(8):
                        nc.vector.tensor_scalar(out=dst[:, fc, cs],
                                                in0=dst[:, fc, cs],
                                                scalar1=gcol[:, fc:fc + 1],
                                                scalar2=bcol[:, fc:fc + 1],
                                                op0=ALU.mult, op1=ALU.add)

            def resid_half(hs, ari):
                hi = 0 if hs.start == 0 else 1
                nc.vector.tensor_tensor(
                    out=h_t[:, :, hs], in0=h_t[:, :, hs],
                    in1=ari[:, hi, :, :], op=ALU.add)

            def gemm_ps(w_t, src, cs, nout, nk, wsr=None, murow=None):
                """Wide-PSUM gemm over token slice cs. With wsr/murow, the
                LN-commute outer products (-wsum x mu) close each oc group
                after all W chains, so stats never gate the W matmuls."""
                ps = pmm.tile([128, 1024], F32, tag="mm")
                w = cs.stop - cs.start
                for oc in range(nout):
                    for kc in range(nk):
                        last = (kc == nk - 1) and wsr is None
                        nc.tensor.matmul(ps[:, oc * w:(oc + 1) * w],
                                         w_t[:, kc, oc * 128:(oc + 1) * 128],
                                         src[:, kc, cs],
                                         start=(kc == 0), stop=last)
                    if wsr is not None:
                        nc.tensor.matmul(ps[:, oc * w:(oc + 1) * w],
                                         wsr[0:1, oc * 128:(oc + 1) * 128],
                                         murow[:], start=False, stop=True)
                return ps

            def drain_ln(ps, out_ap, nout, cs, rsig_b, bias_r):
                """out = ps * rsig_b + bias (rsig broadcast across chunks)."""
                w = cs.stop - cs.start
                bsh = [128, nout, w]
                nc.vector.tensor_tensor(out=out_ap, in0=ps[:, 0:nout * w],
                                        in1=rsig_b.broadcast_to(bsh),
                                        op=ALU.mult)
                nc.vector.tensor_tensor(out=out_ap, in0=out_ap,
                                        in1=bias_r, op=ALU.add)

            def launch_ar(tag, src, hs):
                hi = 0 if hs.start == 0 else 1
                bin_ = dp.tile([128, 1024], FP16, tag=f"i{tag}")
                bout = dp.tile([128, 1024], FP16, tag=f"o{tag}")
                nc.sync.dma_start(bin_[:, 0:512], src[:, hi, 0:4, :])
                nc.sync.dma_start(bin_[:, 512:1024], src[:, hi, 4:8, :])
                nc.gpsimd.collective_compute(
                    "AllReduce", ALU.add, replica_groups=RG4,
                    ins=[bin_[:].opt()], outs=[bout[:].opt()])
                return bout

            def land_ar(bout, dst, hs):
                hi = 0 if hs.start == 0 else 1
                nc.sync.dma_start(dst[:, hi, 0:4, :], bout[:, 0:512])
                nc.sync.dma_start(dst[:, hi, 4:8, :], bout[:, 512:1024])

            def attn_front(hs, l, lb_t, wqkv_t, wsr_t):
                """resid + lazy-LN qkv for one stream."""
                if l > 0:
                    resid_half(hs, ari2_t)
                rmb = stats_half(h_t, hs)
                ps = gemm_ps(wqkv_t, h_t, hs, 6, 8,
                             wsr=wsr_t[0:1, 0:768], murow=rm16_t[0:1, 1, :])
                drain_ln(ps, qkv_t[:, :, hs], 6, hs, rmb[:, 0:1, :],
                         lb_t[:, 0:6, :])

            def attn_A():
                """Heads attend within tokens 0:128 (kt=0 block only)."""
                nc.sync.dma_start(qkvh2_t[:, :, T0], qkv_t[64:128, :, T0])
                for hh in (0, 2, 1, 3):
                    rq = 64 * (hh % 2)
                    src = qkv_t if rq == 0 else qkvh2_t
                    qc, kc2, vc = hh // 2, 2 + hh // 2, 4 + hh // 2
                    ps_s = pat.tile([128, 512], F32, tag="att")
                    nc.tensor.matmul(ps_s[:, 0:128],
                                     src[0:64, kc2, 0:128],
                                     src[0:64, qc, 0:128],
                                     start=True, stop=True)
                    em = tp.tile([128, 512], FP16, tag="em")
                    nc.scalar.activation(em[:, 0:128], ps_s[:, 0:128],
                                         AF.Exp, scale=0.125)
                    nc.vector.tensor_tensor(out=em[:, 0:128], in0=em[:, 0:128],
                                            in1=masks_t[:, 0:128],
                                            op=ALU.mult)
                    ps_d = pst.tile([1, 256], F32, tag="stat")
                    nc.tensor.matmul(ps_d[:, 0:128], ones_t[:, 0:1],
                                     em[:, 0:128], start=True, stop=True)
                    rec = tps.tile([1, 256], F32, tag="rec")
                    nc.vector.reciprocal_approx_fast(out=rec[:, 0:128],
                                                     in_=ps_d[:, 0:128])
                    rec16 = tps.tile([1, 256], FP16, tag="rec16")
                    nc.vector.tensor_copy(rec16[:, 0:128], rec[:, 0:128])
                    ps_rb = pat.tile([128, 512], F32, tag="att")
                    nc.tensor.matmul(ps_rb[:, 0:128], ones_t[0:1, :],
                                     rec16[:, 0:128], start=True, stop=True)
                    rec_b = tp.tile([128, 256], FP16, tag="recb")
                    nc.vector.tensor_copy(rec_b[:, 0:128], ps_rb[:, 0:128])
                    ps_t = ptr.tile([128, 128], FP16, tag="ptr")
                    nc.tensor.transpose(ps_t[:, 0:64],
                                        src[0:64, vc, 0:128],
                                        ident_t[0:64, 0:64])
                    nc.vector.tensor_copy(vtok_t[:, hh, 0, :], ps_t[:, 0:64])
                    ps_o = pat.tile([128, 512], F32, tag="att")
                    nc.tensor.matmul(ps_o[0:64, 0:128], vtok_t[:, hh, 0, :],
                                     em[:, 0:128], start=True, stop=True)
                    if rq == 0:
                        nc.vector.tensor_tensor(out=A_t[0:64, hh // 2, T0],
                                                in0=ps_o[0:64, 0:128],
                                                in1=rec_b[0:64, 0:128],
                                                op=ALU.mult)
                    else:
                        oh = tp.tile([64, 256], FP16, tag="oh")
                        nc.vector.tensor_tensor(out=oh[:, 0:128],
                                                in0=ps_o[0:64, 0:128],
                                                in1=rec_b[0:64, 0:128],
                                                op=ALU.mult)
                        nc.sync.dma_start(A_t[64:128, hh // 2, T0],
                                          oh[:, 0:128])

            def attn_B():
                """Queries 128:256 attend to keys 0:256 (mask on kt=1)."""
                nc.sync.dma_start(qkvh2_t[:, :, T1], qkv_t[64:128, :, T1])
                for hh in (0, 2, 1, 3):
                    rq = 64 * (hh % 2)
                    src = qkv_t if rq == 0 else qkvh2_t
                    qc, kc2, vc = hh // 2, 2 + hh // 2, 4 + hh // 2
                    ps_s = pat.tile([128, 512], F32, tag="att")
                    for kt in range(2):
                        nc.tensor.matmul(
                            ps_s[:, kt * 128:(kt + 1) * 128],
                            src[0:64, kc2, kt * 128:(kt + 1) * 128],
                            src[0:64, qc, T1], start=True, stop=True)
                    em = tp.tile([128, 512], FP16, tag="em")
                    nc.scalar.activation(em[:, 0:256], ps_s[:, 0:256],
                                         AF.Exp, scale=0.125)
                    nc.vector.tensor_tensor(out=em[:, 128:256],
                                            in0=em[:, 128:256],
                                            in1=masks_t[:, 384:512],
                                            op=ALU.mult)
                    ps_d = pst.tile([1, 256], F32, tag="stat")
                    for kt in range(2):
                        nc.tensor.matmul(ps_d[:, 0:128], ones_t[:, 0:1],
                                         em[:, kt * 128:(kt + 1) * 128],
                                         start=(kt == 0), stop=(kt == 1))
                    rec = tps.tile([1, 256], F32, tag="rec")
                    nc.vector.reciprocal_approx_fast(out=rec[:, 0:128],
                                                     in_=ps_d[:, 0:128])
                    rec16 = tps.tile([1, 256], FP16, tag="rec16")
                    nc.vector.tensor_copy(rec16[:, 0:128], rec[:, 0:128])
                    ps_rb = pat.tile([128, 512], F32, tag="att")
                    nc.tensor.matmul(ps_rb[:, 0:128], ones_t[0:1, :],
                                     rec16[:, 0:128], start=True, stop=True)
                    rec_b = tp.tile([128, 256], FP16, tag="recb")
                    nc.vector.tensor_copy(rec_b[:, 0:128], ps_rb[:, 0:128])
                    ps_t = ptr.tile([128, 128], FP16, tag="ptr")
                    nc.tensor.transpose(ps_t[:, 0:64],
                                        src[0:64, vc, T1],
                                        ident_t[0:64, 0:64])
                    nc.vector.tensor_copy(vtok_t[:, hh, 1, :], ps_t[:, 0:64])
                    ps_o = pat.tile([128, 512], F32, tag="att")
                    for kt in range(2):
                        nc.tensor.matmul(ps_o[0:64, 0:128],
                                         vtok_t[:, hh, kt, :],
                                         em[:, kt * 128:(kt + 1) * 128],
                                         start=(kt == 0), stop=(kt == 1))
                    if rq == 0:
                        nc.vector.tensor_tensor(out=A_t[0:64, hh // 2, T1],
                                                in0=ps_o[0:64, 0:128],
                                                in1=rec_b[0:64, 0:128],
                                                op=ALU.mult)
                    else:
                        oh = tp.tile([64, 256], FP16, tag="oh")
                        nc.vector.tensor_tensor(out=oh[:, 0:128],
                                                in0=ps_o[0:64, 0:128],
                                                in1=rec_b[0:64, 0:128],
                                                op=ALU.mult)
                        nc.sync.dma_start(A_t[64:128, hh // 2, T1],
                                          oh[:, 0:128])

            def wo_ar(hs, l, lb_t, wo_t):
                hi = 0 if hs.start == 0 else 1
                ps = gemm_ps(wo_t, A_t, hs, 8, 2)
                nc.vector.tensor_tensor(out=aro_t[:, hi, :, :], in0=ps[:],
                                        in1=lb_t[:, 6:14, :], op=ALU.add)
                return launch_ar(f"ah{hi}", aro_t, hs)

            def mlp(hs, l, lb_t, w1_t, w2_t, wsr_t, ar_a):
                hi = 0 if hs.start == 0 else 1
                land_ar(ar_a, ari_t, hs)
                resid_half(hs, ari_t)
                rmb = stats_half(h_t, hs)
                ps = gemm_ps(w1_t, h_t, hs, 8, 8,
                             wsr=wsr_t[0:1, 768:1792], murow=rm16_t[0:1, 1, :])
                w1t = tp.tile([128, 8, 128], FP16, tag="w1t")
                drain_ln(ps, w1t[:], 8, hs, rmb[:, 0:1, :],
                         lb_t[:, 22:30, :])
                nc.scalar.activation(gu_t[:, :, hs], w1t[:], AF.Gelu)
                ps2 = gemm_ps(w2_t, gu_t, hs, 8, 8)
                nc.vector.tensor_tensor(out=aro2_t[:, hi, :, :], in0=ps2[:],
                                        in1=lb_t[:, 14:22, :], op=ALU.add)
                return launch_ar(f"mh{hi}", aro2_t, hs)

            # ---------------- patch pooling: h = patchesT ----------------
            for hs in (T0, T1):
                ps = pmm.tile([128, 1024], F32, tag="mm")
                for oc in range(8):
                    for kc in range(2):
                        nc.tensor.matmul(
                            ps[:, oc * 128:(oc + 1) * 128],
                            embS_t[:, kc, oc * 128:(oc + 1) * 128],
                            cnt_t[:, kc, hs],
                            start=(kc == 0), stop=(kc == 1))
                nc.vector.tensor_copy(h_t[:, :, hs], ps[:])

            # ---------------- transformer layers ----------------
            ar_m = [None, None]
            for l in range(L):
                wqkv_t = wp.tile([128, 8, 768], FP16, tag="wqkv")
                for q in range(8):
                    nc.sync.dma_start(
                        wqkv_t[:, q, :],
                        d["wqkv"][l].rearrange("p (kc x) -> p kc x", kc=8)
                        [:, q, :])
                wo_t = wp.tile([128, 2, 1024], FP16, tag="wo")
                for q in range(2):
                    nc.sync.dma_start(wo_t[:, q, :], d["wo"][l].rearrange(
                        "p (kc x) -> p kc x", kc=2)[:, q, :])
                w1_t = wp.tile([128, 8, 1024], FP16, tag="w1")
                for q in range(8):
                    nc.sync.dma_start(
                        w1_t[:, q, :],
                        d["w1"][l].rearrange("p (kc x) -> p kc x", kc=8)
                        [:, q, :])
                w2_t = wp.tile([128, 8, 1024], FP16, tag="w2")
                for q in range(8):
                    nc.sync.dma_start(
                        w2_t[:, q, :],
                        d["w2"][l].rearrange("p (kc x) -> p kc x", kc=8)
                        [:, q, :])
                lb_t = wp.tile([128, 30, 128], FP16, tag="lb", bufs=1)
                for q in range(2):
                    nc.sync.dma_start(
                        lb_t[:, q * 15:(q + 1) * 15, :],
                        d["lb"][l].rearrange("p (kc x) -> p kc x", kc=30)
                        [:, q * 15:(q + 1) * 15, :])
                wsr_t = wp.tile([1, 1792], FP16, tag="wsr", bufs=1)
                nc.sync.dma_start(wsr_t[:], d["wsrow"][l])

                # stream A attention sublayer, AR flies under stream B front
                attn_front(T0, l, lb_t, wqkv_t, wsr_t)
                attn_A()
                ar_a0 = wo_ar(T0, l, lb_t, wo_t)

                # stream B attention sublayer
                attn_front(T1, l, lb_t, wqkv_t, wsr_t)
                attn_B()
                ar_a1 = wo_ar(T1, l, lb_t, wo_t)

                # byte-path filler (independent of ARs)
                if l == 0:
                    for hs in (T0, T1):
                        rmb = stats_half(embT_t, hs)
                        nc.vector.tensor_copy(qnr_t[:, hs], rmb[:, 0, :])
                        nc.vector.tensor_copy(qnmu_t[0:1, hs],
                                              rm16_t[0:1, 1, :])
                elif l == 1:
                    ps = pat.tile([128, 512], F32, tag="att")
                    for oc in range(2):
                        for kc in range(8):
                            nc.tensor.matmul(
                                ps[:, oc * 256:(oc + 1) * 256],
                                wq_t[:, kc, oc * 128:(oc + 1) * 128],
                                embT_t[:, kc, :],
                                start=(kc == 0), stop=False)
                        nc.tensor.matmul(
                            ps[:, oc * 256:(oc + 1) * 256],
                            cawsrow_t[0:1, oc * 128:(oc + 1) * 128],
                            qnmu_t[:], start=False, stop=True)
                    nc.vector.tensor_tensor(
                        out=qT_t[:], in0=ps[:],
                        in1=qnr_t[:].unsqueeze(1).broadcast_to([128, 2, 256]),
                        op=ALU.mult)
                    nc.vector.tensor_tensor(out=qT_t[:], in0=qT_t[:],
                                            in1=cabR_t[:, 0:2, :],
                                            op=ALU.add)
                elif l == 2:
                    ps = pat.tile([128, 512], F32, tag="att")
                    for oc in range(2):
                        for kc in range(8):
                            nc.tensor.matmul(
                                ps[:, oc * 256:(oc + 1) * 256],
                                headw_t[:, kc, oc * 128:(oc + 1) * 128],
                                embT_t[:, kc, :],
                                start=(kc == 0), stop=(kc == 7))
                    nc.vector.tensor_copy(et_t[:], ps[:])
                else:
                    ps = pat.tile([128, 512], F32, tag="att")
                    for oc in range(2):
                        for kc in range(8):
                            nc.tensor.matmul(
                                ps[:, oc * 256:(oc + 1) * 256],
                                cawoT_t[:, kc, oc * 128:(oc + 1) * 128],
                                headw_t[:, kc, :],
                                start=(kc == 0), stop=(kc == 7))
                    nc.vector.tensor_copy(w2c_t[:], ps[:])

                # mlp sublayers: A then B; each AR hides under the next block
                ar_m[0] = mlp(T0, l, lb_t, w1_t, w2_t, wsr_t, ar_a0)
                ar_m[1] = mlp(T1, l, lb_t, w1_t, w2_t, wsr_t, ar_a1)
                for hi, hs in enumerate((T0, T1)):
                    land_ar(ar_m[hi], ari2_t, hs)

            # ---------------- final: resid + fn/ca norm -> kvn ----------
            if skip_kvn_ln:
                kvsrc_t = h_t
                for hs in (T0, T1):
                    resid_half(hs, ari2_t)
                    rmb = stats_half(h_t, hs)
                    nc.vector.tensor_copy(ffr_t[:, hs], rmb[:, 0, :])
                    nc.vector.tensor_copy(fmu_t[0:1, hs], rm16_t[0:1, 1, :])
            else:
                pf_t = gu_t
                kvsrc_t = pf_t
                for hs in (T0, T1):
                    resid_half(hs, ari2_t)
                    rmb = stats_half(h_t, hs)
                    norm_half(h_t, pf_t, hs, rmb,
                              gcol=cbias_t[:, 2:10], bcol=cbias_t[:, 10:18])
                for hs in (T0, T1):
                    rmb = stats_half(pf_t, hs)
                    nc.vector.tensor_copy(ffr_t[:, hs], rmb[:, 0, :])
                    nc.vector.tensor_copy(fmu_t[0:1, hs], rm16_t[0:1, 1, :])

            # ---------------- CA k/v projections (lazy LN) ----------------
            for (w_t, out_t, bc0, ws0) in ((wk_t, kT_t, 2, 256),
                                           (wv_t, vT_t, 4, 512)):
                ps = pat.tile([128, 512], F32, tag="att")
                for oc in range(2):
                    for kc in range(8):
                        nc.tensor.matmul(
                            ps[:, oc * 256:(oc + 1) * 256],
                            w_t[:, kc, oc * 128:(oc + 1) * 128],
                            kvsrc_t[:, kc, :], start=(kc == 0), stop=False)
                    nc.tensor.matmul(
                        ps[:, oc * 256:(oc + 1) * 256],
                        cawsrow_t[0:1, ws0 + oc * 128:ws0 + (oc + 1) * 128],
                        fmu_t[:], start=False, stop=True)
                nc.vector.tensor_tensor(
                    out=out_t[:], in0=ps[:],
                    in1=ffr_t[:].unsqueeze(1).broadcast_to([128, 2, 256]),
                    op=ALU.mult)
                nc.vector.tensor_tensor(out=out_t[:], in0=out_t[:],
                                        in1=cabR_t[:, bc0:bc0 + 2, :],
                                        op=ALU.add)

            # ---------------- CA attention (2 heads, dh=128) ----------
            for chh in range(2):
                ps_s = pat.tile([128, 512], F32, tag="att")
                for kt in range(2):
                    nc.tensor.matmul(
                        ps_s[:, kt * 256:(kt + 1) * 256],
                        kT_t[:, chh, kt * 128:(kt + 1) * 128],
                        qT_t[:, chh, :], start=True, stop=True)
                em = tp.tile([128, 512], FP16, tag="em")
                nc.scalar.activation(em[:], ps_s[:], AF.Exp,
                                     scale=float(1.0 / np.sqrt(128.0)))
                ps_d = pst.tile([1, 256], F32, tag="stat")
                for kt in range(2):
                    nc.tensor.matmul(ps_d[:], ones_t[:, 0:1],
                                     em[:, kt * 256:(kt + 1) * 256],
                                     start=(kt == 0), stop=(kt == 1))
                rec = tps.tile([1, 256], F32, tag="rec")
                nc.vector.reciprocal_approx_fast(out=rec[:], in_=ps_d[:])
                rec16 = tps.tile([1, 256], FP16, tag="rec16")
                nc.vector.tensor_copy(rec16[:], rec[:])
                ps_rb = pat.tile([128, 512], F32, tag="att")
                nc.tensor.matmul(ps_rb[:, 0:256], ones_t[0:1, :], rec16[:],
                                 start=True, stop=True)
                rec_b = tp.tile([128, 256], FP16, tag="recb")
                nc.vector.tensor_copy(rec_b[:], ps_rb[:, 0:256])
                vtokca = tp.tile([128, 2, 128], FP16, tag="vtokca")
                for kt in range(2):
                    ps_t = ptr.tile([128, 128], FP16, tag="ptr")
                    nc.tensor.transpose(
                        ps_t[:], vT_t[:, chh, kt * 128:(kt + 1) * 128],
                        ident_t[:])
                    nc.vector.tensor_copy(vtokca[:, kt, :], ps_t[:])
                ps_o = pat.tile([128, 512], F32, tag="att")
                for kt in range(2):
                    nc.tensor.matmul(ps_o[:, 0:256], vtokca[:, kt, :],
                                     em[:, kt * 256:(kt + 1) * 256],
                                     start=(kt == 0), stop=(kt == 1))
                nc.vector.tensor_tensor(out=O_t[:, chh, :],
                                        in0=ps_o[:, 0:256],
                                        in1=rec_b[:], op=ALU.mult)

            # ---------------- logits partials + AR (fp16) ----------------
            lp_t = sbp.tile([128, 2, 256], FP16, tag="lp")
            ps = pat.tile([128, 512], F32, tag="att")
            for vo in range(2):
                for od in range(2):
                    nc.tensor.matmul(ps[:, vo * 256:(vo + 1) * 256],
                                     w2c_t[:, od, vo * 128:(vo + 1) * 128],
                                     O_t[:, od, :],
                                     start=(od == 0), stop=(od == 1))
            nc.vector.tensor_copy(lp_t[:], ps[:])
            lbin = dp.tile([128, 512], FP16, tag="lci")
            lbout = dp.tile([128, 512], FP16, tag="lco")
            nc.sync.dma_start(lbin[:], lp_t[:])
            nc.gpsimd.collective_compute(
                "AllReduce", ALU.add, replica_groups=RG4,
                ins=[lbin[:].opt()], outs=[lbout[:].opt()])
            lar_t = sbp.tile([128, 2, 256], FP16, tag="lar")
            nc.sync.dma_start(lar_t[:], lbout[:])

            out_t = sbp.tile([128, 2, 256], F32, tag="outt")
            for vo in range(2):
                nc.vector.tensor_scalar(out=out_t[:, vo, :],
                                        in0=lar_t[:, vo, :],
                                        scalar1=cbias_t[:, vo:vo + 1],
                                        scalar2=None, op0=ALU.add)
                nc.vector.tensor_tensor(out=out_t[:, vo, :],
                                        in0=out_t[:, vo, :],
                                        in1=et_t[:, vo, :], op=ALU.add)
            nc.sync.dma_start(out_d[:], out_t[:])

    nc.compile()
    nc.m = get_hw_module(nc.m)
    return nc


# --------------------------------------------------------------------------
# host side
# --------------------------------------------------------------------------
def _shuf16(M):
    """[K, X] -> [128, (K//128)*X] fp16 laid out as [p, kc, x]."""
    K, X = M.shape
    return np.ascontiguousarray(
        M.reshape(K // 128, 128, X).transpose(1, 0, 2).reshape(128, -1)
    ).astype(np.float16)


def _rep(bias, nc_, w):
    """bias [nc_*128] -> [128, nc_, w] fp16 replicated along tokens."""
    return np.broadcast_to(
        bias.reshape(nc_, 128).T[:, :, None], (128, nc_, w)
    ).astype(np.float16)


def _prep(inputs):
    f = lambda k: np.asarray(inputs[k], np.float32)
    byte_seq = np.asarray(inputs["byte_seq"])
    bd = np.asarray(inputs["patch_boundaries"])
    emb = f("emb")

    pos = np.arange(S)
    pid = np.stack([np.searchsorted(bd[b], pos, side="right")
                    for b in range(B)])
    pid = np.clip(pid, 0, P - 1)
    Cn = np.zeros((B, P, V), np.float32)
    for b in range(B):
        np.add.at(Cn[b], (pid[b], byte_seq[b]), 1.0)
    cnts = Cn.sum(-1)
    Cn /= np.maximum(cnts, 1.0)[..., None]

    g1, b1a = f("g_ln1_g"), f("g_ln1_b")
    g2, b2a = f("g_ln2_g"), f("g_ln2_b")
    Wqkv, bqkv = f("g_wqkv"), f("g_bqkv")
    Wo, bo = f("g_wo"), f("g_bo")
    W1, b1 = f("g_w1"), f("g_b1")
    W2, b2 = f("g_w2"), f("g_b2")

    Wq_f = g1[:, :, None] * Wqkv
    biasq = np.einsum("lh,lho->lo", b1a, Wqkv) + bqkv
    W1_f = g2[:, :, None] * W1
    bias1 = np.einsum("lh,lho->lo", b2a, W1) + b1

    ca_wqkv, ca_bqkv = f("ca_wqkv"), f("ca_bqkv")
    ca_wo, ca_bo = f("ca_wo"), f("ca_bo")
    head_w, head_b = f("head_w"), f("head_b")
    cag, cab = f("ca_ln_g"), f("ca_ln_b")
    headb_full = head_b + ca_bo @ head_w

    wq_e = cag[:, None] * ca_wqkv[:, :H]
    bq_e = cab @ ca_wqkv[:, :H] + ca_bqkv[:H]
    wk_e = cag[:, None] * ca_wqkv[:, H:2 * H]
    bk_e = cab @ ca_wqkv[:, H:2 * H] + ca_bqkv[H:2 * H]
    wv_e = cag[:, None] * ca_wqkv[:, 2 * H:]
    bv_e = cab @ ca_wqkv[:, 2 * H:] + ca_bqkv[2 * H:]

    masks = np.zeros((128, 2, 256), np.float32)
    for kt in range(2):
        ktg = kt * 128 + np.arange(128)
        masks[:, kt, :] = (ktg[:, None] <= np.arange(256)[None, :])

    in_maps = []
    for c in range(NC):
        b, r = c // 4, c % 4
        m = {}
        cols = np.concatenate([np.arange(r * 256, (r + 1) * 256) + k * H
                               for k in range(3)])
        m["wqkv"] = np.stack([_shuf16(Wq_f[l][:, cols]) for l in range(L)])
        m["wo"] = np.stack([_shuf16(Wo[l][r * 256:(r + 1) * 256, :])
                            for l in range(L)])
        m["w1"] = np.stack(
            [_shuf16(W1_f[l][:, r * 1024:(r + 1) * 1024]) for l in range(L)])
        m["w2"] = np.stack(
            [_shuf16(W2[l][r * 1024:(r + 1) * 1024, :]) for l in range(L)])
        lb = np.zeros((L, 128, 30, 128), np.float16)
        wsrow = np.zeros((L, 1, 1792), np.float32)
        for l in range(L):
            lb[l, :, 0:6, :] = _rep(biasq[l, cols], 6, 128)
            lb[l, :, 6:14, :] = _rep(bo[l] / 4, 8, 128)
            lb[l, :, 14:22, :] = _rep(b2[l] / 4, 8, 128)
            lb[l, :, 22:30, :] = _rep(bias1[l, r * 1024:(r + 1) * 1024],
                                      8, 128)
            wsrow[l, 0, 0:768] = -Wq_f[l][:, cols].sum(0)
            wsrow[l, 0, 768:1792] = -W1_f[l][:, r * 1024:(r + 1) * 1024].sum(0)
        m["lb"] = np.ascontiguousarray(lb.reshape(L, 128, 3840))
        m["wsrow"] = wsrow.astype(np.float16)
        m["cawsrow"] = np.concatenate([
            -wq_e[:, r * 256:(r + 1) * 256].sum(0),
            -wk_e[:, r * 256:(r + 1) * 256].sum(0),
            -wv_e[:, r * 256:(r + 1) * 256].sum(0)])[None, :].astype(
            np.float16)
        cabR = np.concatenate([
            _rep(bq_e[r * 256:(r + 1) * 256], 2, 256),
            _rep(bk_e[r * 256:(r + 1) * 256], 2, 256),
            _rep(bv_e[r * 256:(r + 1) * 256], 2, 256)], axis=1)
        m["cabR"] = np.ascontiguousarray(cabR.reshape(128, 1536))
        m["cnt"] = _shuf16(Cn[b].T)
        m["embS"] = _shuf16(emb)
        m["embT"] = _shuf16(np.ascontiguousarray(emb.T))
        m["masks"] = np.ascontiguousarray(
            masks.reshape(128, 512)).astype(np.float16)
        m["ones"] = np.ones((128, 128), np.float16)
        m["ident"] = np.eye(128, dtype=np.float16)
        m["wq"] = _shuf16(wq_e[:, r * 256:(r + 1) * 256])
        m["wk"] = _shuf16(wk_e[:, r * 256:(r + 1) * 256])
        m["wv"] = _shuf16(wv_e[:, r * 256:(r + 1) * 256])
        m["cawoT"] = _shuf16(np.ascontiguousarray(
            ca_wo[r * 256:(r + 1) * 256, :].T))
        m["headw"] = _shuf16(head_w)
        cbias = np.zeros((128, 18), np.float32)
        cbias[:, 0:2] = headb_full.reshape(2, 128).T
        cbias[:, 2:10] = f("fn_g").reshape(8, 128).T
        cbias[:, 10:18] = f("fn_b").reshape(8, 128).T
        m["cbias"] = np.ascontiguousarray(cbias)
        in_maps.append(m)
    return in_maps, byte_seq


def run_device(inputs, trace=False):
    skip = (np.allclose(np.asarray(inputs["fn_g"]), 1.0)
            and np.allclose(np.asarray(inputs["fn_b"]), 0.0))
    key = ("nc", skip)
    if key not in _CACHE:
        _CACHE[key] = _trace(skip)
    nc = _CACHE[key]
    in_maps, byte_seq = _prep(inputs)
    res = run_bass_kernel_spmd(nc, in_maps, core_ids=list(range(NC)),
                               trace=trace)
    out = np.empty((B, S, V), np.float32)
    for b in range(B):
        ltab = res.results[b * 4]["ltab"]
        tab = ltab.reshape(128, 2, 256).transpose(1, 0, 2).reshape(256, 256)
        out[b] = tab.T[byte_seq[b]]
    return out, res


def kernel(**inputs) -> np.ndarray:
    out, _ = run_device(inputs, trace=False)
    return out
